# revision 33
# baseline (speedup 1.0000x reference)
"""Trainium2 Bass kernel for nn_CloMSFM (CloFormer-style mixed local-conv +
global-attention block). Data-parallel over batch: 16 images -> 8 NeuronCores,
2 images per core, no collectives.

v4 design (vs. v3):
  - DMA diet: dw-conv diagonal weight matrices built on-device (identity x
    per-channel scalar on the GpSimd engine) instead of an 884KB DMA; xf
    residual shipped bf16; proj weights bf16. Input DMAs ordered x8 first,
    xf last -> compute starts ~4us in instead of ~16us.
  - S phase runs p-major (head-pair-major) so each image's first head pair
    finishes early; V is split per head-pair and pipelined into the weave
    windows (V(0) p0 in W1, V(0) p1 + V(1) p0 in W2, only V(1) p1 + P(1)
    in the tail).
  - time-paced weave with finer filler granularity (per-tap-pair yields).
  - softmax denominator: reciprocal from psum partition 0 (den column 0 of
    the numerator lhsT), GpSimd partition_broadcast via base-0 tiles.
"""

import os
import sys

for _p in ("/opt/trn_rl_repo",):
    if os.path.isdir(_p) and _p not in sys.path:
        sys.path.insert(0, _p)

import numpy as np
import ml_dtypes

BF16 = ml_dtypes.bfloat16
F8 = ml_dtypes.float8_e4m3fn

DIM = 256
DIM_HEAD = 32
SCALOR = DIM_HEAD ** (-0.5)
HF_CH = 128
QKV_CH = 384
LF_HEADS = 4
HW = 1024
B_PER_CORE = 2
N_CORES = 8

# fp8 weight bundle [128, W8] column offsets
OF_QKV = 0            # 2*384: [kc2, blk*128+m], x16
OF_GQ = 768           # 2*128 x16
OF_GK = 1024          # 2*128 x16
OF_GV = 1280          # 2*128 x16
W8 = 1536

# bf16 bundle [128, WB]
OF_A1 = 0             # 128   (act_w1/256).T
OF_A2 = 128           # 128   (0.5*act_w2).T
OF_CA1 = 256          # 32    (ca_w1/(1024*16)).T
OF_CA2 = 288          # 256   rows 0-15: ca_w2.T
OF_GVB = 544          # 128   row 0: 16*gkv_b[v]
OF_PJ = 672           # 512   (0.5*proj_w).T
WB = 1184

# f32 bundle [128, WF]
OF_QKVB = 0           # 3  qkv_b (true scale; psum is x16, mult 1/16 first)
OF_DWB = 3            # 3  16*dw_b
OF_B1H = 6            # 1  0.5*act_b1
OF_B2S = 7            # 1  SCALOR*act_b2
OF_GQB = 8            # 1  gq_b
OF_GKB = 9            # 1  gkv_b[k]
OF_B1 = 10            # 1  act_b1
WF = 11

_CACHE = {}


def _build_nc():
    import concourse.tile as tile
    from concourse import mybir, bacc

    f32 = mybir.dt.float32
    bf16 = mybir.dt.bfloat16
    f8 = mybir.dt.float8e4
    AF = mybir.ActivationFunctionType
    OP = mybir.AluOpType
    DR = mybir.MatmulPerfMode.DoubleRow

    nc = bacc.Bacc("TRN2", target_bir_lowering=False, debug=False,
                   num_devices=N_CORES, enable_asserts=True)

    x8_d = nc.dram_tensor("x8", [2, 128, 2, HW], f8, kind="ExternalInput").ap()
    xf_d = nc.dram_tensor("xf", [2, 2, 128, HW], bf16, kind="ExternalInput").ap()
    w8_d = nc.dram_tensor("w8", [128, W8], f8, kind="ExternalInput").ap()
    wb_d = nc.dram_tensor("wb", [128, WB], bf16, kind="ExternalInput").ap()
    wf_d = nc.dram_tensor("wf", [128, WF], f32, kind="ExternalInput").ap()
    dwd_d = nc.dram_tensor("dwd", [128, 27, 128], bf16, kind="ExternalInput").ap()
    out_d = nc.dram_tensor("out", [2, 2, 128, HW], bf16, kind="ExternalOutput").ap()

    from contextlib import ExitStack
    with tile.TileContext(nc) as tc, ExitStack() as ctx:
        cw = ctx.enter_context(tc.tile_pool(name="cw", bufs=1))
        sb = ctx.enter_context(tc.tile_pool(name="sb", bufs=2))
        su = ctx.enter_context(tc.tile_pool(name="su", bufs=2))
        ps = ctx.enter_context(tc.tile_pool(name="ps", bufs=2, space="PSUM"))

        st = [dict() for _ in range(2)]
        cwt = {}

        def phase_x8(img):
            x8 = sb.tile([128, 2, HW], f8, name="x8", tag="x8")
            st[img]["x8"] = x8
            if img == 1:
                # gate the transfer behind early conv(0) output so it does
                # not compete with the startup DMA burst (WAR dep on the
                # 1-element marker write below)
                nc.vector.tensor_copy(x8[0:1, 0, 0:1], st[0]["q_sb"][0:1, 0:1])
            nc.sync.dma_start(x8[:, :, :], x8_d[img])
            pf = sb.tile([128, 2], f32, name="pf", tag="pf")
            st[img]["pf"] = pf
            yield 100

        def phase_w1():
            wf = cw.tile([128, WF], f32, name="wf", tag="wf")
            nc.sync.dma_start(wf[:, :], wf_d[:, :])
            w8 = cw.tile([128, W8], f8, name="w8", tag="w8")
            nc.sync.dma_start(w8[:, :], w8_d[:, :])
            wb = cw.tile([128, WB], bf16, name="wb", tag="wb")
            nc.sync.dma_start(wb[:, :], wb_d[:, :])
            # gv bias broadcast straight from DRAM (replicating DMA read)
            bb = cw.tile([128, 128], bf16, name="bb", tag="bb")
            nc.sync.dma_start(
                bb[:, :], wb_d[0:1, OF_GVB:OF_GVB + 128].to_broadcast((128, 128)))
            cwt.update(w8=w8, wf=wf, wb=wb, bb=bb)
            yield 100

        def phase_wd():
            dwd = cw.tile([128, 27, 128], bf16, name="dwd", tag="dwd")
            nc.sync.dma_start(dwd[:, :, :], dwd_d[:, :, :])
            cwt.update(dwd=dwd)
            yield 100

        def w8kc(off, width, m0, msz):  # [128, 2, msz] DR lhsT view into w8
            return cwt["w8"][:, off:off + 2 * width].rearrange(
                "p (k m) -> p k m", k=2)[:, :, m0:m0 + msz]

        def bias(off):
            return cwt["wf"][:, off:off + 1]

        def phase_qk(img):
            """gq/gk 1x1 convs -> qT/kT (bf16); unblocks phase_S(img)."""
            s = st[img]
            x8 = s["x8"]
            gp = ps.tile([128, 2, 512], f32, name="mm", tag="mm")
            for nh in range(2):
                nc.tensor.matmul(gp[:, nh, :], w8kc(OF_GQ, 128, 0, 128),
                                 x8[:, :, nh * 512:nh * 512 + 512],
                                 start=True, stop=True, perf_mode=DR)
            qT = sb.tile([128, HW], bf16, name="qT", tag="qT")
            s["qT"] = qT
            nc.vector.tensor_scalar(qT.rearrange("p (a b) -> p a b", a=2),
                                    gp[:, 0:2, :], 0.0625, bias(OF_GQB),
                                    OP.mult, OP.add)
            yield 800
            gk = ps.tile([128, 2, 512], f32, name="mm", tag="mm")
            for nh in range(2):
                nc.tensor.matmul(gk[:, nh, :], w8kc(OF_GK, 128, 0, 128),
                                 x8[:, :, nh * 512:nh * 512 + 512],
                                 start=True, stop=True, perf_mode=DR)
            kT = sb.tile([128, HW], bf16, name="kT", tag="kT")
            s["kT"] = kT
            nc.vector.tensor_scalar(kT.rearrange("p (a b) -> p a b", a=2),
                                    gk[:, 0:2, :], 0.0625, bias(OF_GKB),
                                    OP.mult, OP.add)
            yield 800

        def phase_xf(img):
            xfc = [sb.tile([128, HW], bf16, name=f"xf{c}", tag=f"xf{c}")
                   for c in range(2)]
            st[img]["xfc"] = xfc
            gate = st[0]["qk"] if img == 0 else st[1]["qT"]
            for c in range(2):
                # gate behind mid-kernel data (WAR dep on the marker write)
                # so the Sync engine can't start these during the startup
                # DMA burst
                nc.vector.tensor_copy(xfc[c][0:1, 0:1], gate[0:1, 0:1])
                nc.sync.dma_start(xfc[c][:, :], xf_d[img, c])
            yield 100

        def phase_gv(img):
            """vatt: [128 tok, kc2(2), jj(4), h(4), 64]; col 0 = 1/16 (den
            trick), cols 32-63 v dims (psum partition alignment)."""
            s = st[img]
            x8 = s["x8"]
            vatt = sb.tile([128, 2, 4, 4, 64], f8, name="vatt", tag="vatt")
            s["vatt"] = vatt
            nc.gpsimd.memset(vatt[:, :, :, :, 0:32], 0.0)
            nc.gpsimd.memset(vatt[:, :, :, :, 0:1], 0.0625)
            yield 100
            for mc in range(8):
                vp = ps.tile([128, 2, 512], f32, name="mm", tag="mm")
                nc.tensor.matmul(vp[:, 0, 0:128],
                                 x8[:, :, mc * 128:mc * 128 + 128],
                                 w8kc(OF_GV, 128, 0, 128),
                                 start=True, stop=True, perf_mode=DR)
                nc.vector.scalar_tensor_tensor(
                    vatt[:, mc % 2, mc // 2, :, 32:64],
                    vp[:, 0, 0:128].rearrange("p (h d) -> p h d", d=32),
                    0.0625,
                    cwt["bb"].rearrange("p (h d) -> p h d", d=32),
                    OP.mult, OP.add)
                yield 420

        def phase_conv(img, blks=(0, 1, 2), acts=True):
            """High-frequency branch: qkv 1x1 (fp8 DR), dw 3x3 (flat bf16
            diag taps), act convs, swish/tanh -> comb[:,0,:] (16x scale)."""
            s = st[img]
            x8, pf = s["x8"], s["pf"]
            if 0 in blks:
                comb = sb.tile([128, 2, HW], f8, name="comb", tag="comb")
                s["comb"] = comb
            comb = s["comb"]
            for blk in blks:
                vpad = sb.tile([128, 1152], bf16, name=f"vpad{blk}",
                               tag=f"vpad{blk}")
                nc.vector.memset(vpad[:, 0:64], 0.0)
                nc.vector.memset(vpad[:, 1088:1152], 0.0)
                cps = ps.tile([128, 2, 512], f32, name="mm", tag="mm")
                for nh in range(2):
                    nc.tensor.matmul(cps[:, nh, :],
                                     w8kc(OF_QKV, 384, blk * 128, 128),
                                     x8[:, :, nh * 512:nh * 512 + 512],
                                     start=True, stop=True, perf_mode=DR)
                nc.vector.tensor_scalar(
                    vpad[:, 64:1088].rearrange("p (a b) -> p a b", a=2),
                    cps[:, 0:2, :], 0.0625, bias(OF_QKVB + blk),
                    OP.mult, OP.add)
                yield 700
                dp = ps.tile([128, 2, 512], f32, name="mm", tag="mm")
                for tap in range(9):
                    sh = 32 * (tap // 3 - 1) + (tap % 3 - 1)
                    for nh in range(2):
                        nc.tensor.matmul(
                            dp[:, nh, :], cwt["dwd"][:, blk * 9 + tap, :],
                            vpad[:, 64 + sh + nh * 512:64 + sh + nh * 512 + 512],
                            start=(tap == 0), stop=(tap == 8))
                    yield 440
                if blk == 0:
                    q_sb = sb.tile([128, HW], bf16, name="q", tag="q")
                    s["q_sb"] = q_sb
                    nc.vector.tensor_scalar_add(
                        q_sb.rearrange("p (a b) -> p a b", a=2),
                        dp[:, 0:2, :], bias(OF_DWB + 0))
                elif blk == 1:
                    qk = sb.tile([128, HW], bf16, name="qk", tag="qk")
                    s["qk"] = qk
                    nc.vector.scalar_tensor_tensor(
                        qk.rearrange("p (a b) -> p a b", a=2),
                        dp[:, 0:2, :], bias(OF_DWB + 1),
                        s["q_sb"].rearrange("p (a b) -> p a b", a=2),
                        OP.add, OP.mult)
                else:
                    v_sb = sb.tile([128, HW], bf16, name="v", tag="v")
                    s["v_sb"] = v_sb
                    nc.vector.tensor_scalar_add(
                        v_sb.rearrange("p (a b) -> p a b", a=2),
                        dp[:, 0:2, :], bias(OF_DWB + 2))
                yield 200

            if not acts:
                return
            qk = s["qk"]
            ap1 = ps.tile([128, 2, 512], f32, name="mm", tag="mm")
            for nh in range(2):
                nc.tensor.matmul(ap1[:, nh, :], cwt["wb"][:, OF_A1:OF_A1 + 128],
                                 qk[:, nh * 512:nh * 512 + 512],
                                 start=True, stop=True)
            yield 440
            # swish via tanh (stays in the exp/tanh/relu act table -> no
            # table reloads between interleaved softmax exps)
            t_sb = sb.tile([128, HW], bf16, name="t", tag="t")
            nc.scalar.activation(t_sb.rearrange("p (a b) -> p a b", a=2),
                                 ap1[:, 0:2, :], AF.Tanh, bias=bias(OF_B1H),
                                 scale=0.5)
            a_sb = sb.tile([128, HW], bf16, name="a", tag="a")
            nc.vector.tensor_scalar_add(
                a_sb.rearrange("p (a b) -> p a b", a=2), ap1[:, 0:2, :],
                bias(OF_B1))
            sw = sb.tile([128, HW], bf16, name="sw", tag="sw")
            nc.vector.scalar_tensor_tensor(sw[:, :], t_sb[:, :], 1.0,
                                           a_sb[:, :], OP.add, OP.mult)
            yield 200
            ap2 = ps.tile([128, 2, 512], f32, name="mm", tag="mm")
            for nh in range(2):
                nc.tensor.matmul(ap2[:, nh, :], cwt["wb"][:, OF_A2:OF_A2 + 128],
                                 sw[:, nh * 512:nh * 512 + 512],
                                 start=True, stop=True)
            yield 440
            th = sb.tile([128, HW], bf16, name="th", tag="th")
            nc.scalar.activation(th.rearrange("p (a b) -> p a b", a=2),
                                 ap2[:, 0:2, :], AF.Tanh, bias=bias(OF_B2S),
                                 scale=SCALOR)
            nc.vector.scalar_tensor_tensor(comb[:, 0, :], th[:, :], 1.0,
                                           s["v_sb"][:, :], OP.mult, OP.mult,
                                           accum_out=pf[:, 0:1])
            yield 200

        def phase_S(img, ps_=(0, 1)):
            """S^T = K^T q per 128-key block; head-PAIR-major (p outer) so
            heads 2p,2p+1 complete after each half; exp -> fp8 U.
            u layout: [128 keys, t(2), jj(4), h(4), nh(2), 512]; j = 2*jj+t."""
            s = st[img]
            if 0 in ps_:
                u = su.tile([128, 2, 4, 4, 2, 512], f8, name="u", tag="u")
                s["u"] = u
            u = s["u"]
            pend = None
            for p in ps_:
                for j in range(8):
                    for nh in range(2):
                        qT, kT = s["qT"], s["kT"]
                        sp = ps.tile([128, 2, 512], f32, name="sp", tag="sp")
                        for hh in range(2):
                            h = 2 * p + hh
                            nc.tensor.matmul(
                                sp[:, hh, :],
                                kT[32 * h:32 * h + 32, j * 128:j * 128 + 128],
                                qT[32 * h:32 * h + 32, nh * 512:nh * 512 + 512],
                                start=True, stop=True,
                                tile_position=(32 * h, 0))
                        # previous unit's exp issues AFTER this unit's mms:
                        # filler chunks between units can no longer delay
                        # the exp's inputs
                        if pend is not None:
                            nc.scalar.activation(*pend, scale=SCALOR)
                        pend = (u[:, j % 2, j // 2, 2 * p:2 * p + 2, nh, :],
                                sp[:, :, :], AF.Exp)
                        yield 1150
            nc.scalar.activation(*pend, scale=SCALOR)

        def _flush_den(img):
            """Deferred normalize: issued one head-group later so the DVE
            stt never head-of-line blocks on the Pool broadcast."""
            s = st[img]
            if "pden" in s:
                vp, dbc, h = s.pop("pden")
                nc.vector.scalar_tensor_tensor(
                    s["comb"][32 * h:32 * h + 32, 1, :],
                    vp[32:64, 0:2, :].rearrange("p a b -> p (a b)"), 1.0,
                    dbc[:, :], OP.mult, OP.mult,
                    accum_out=s["pf"][32 * h:32 * h + 32, 1:2])

        def phase_V(img, heads, fence_base=None, jj_major=False):
            """Numerator via fp8 DR over j-pairs; den column 0 ->
            reciprocal from psum partition 0; GpSimd partition_broadcast;
            DVE normalize into comb (deferred one head).  fence_base: pacer
            unit count after which this pair's first j-block exps exist;
            per-jj fences keep issue order safe."""
            s = st[img]
            vatt, u, comb, pf = s["vatt"], s["u"], s["comb"], s["pf"]
            if jj_major:
                vps = {h: ps.tile([128, 2, 512], f32, name="mm", tag="mm")
                       for h in heads}
                for jj in range(4):
                    if fence_base is not None:
                        yield ('fence', fence_base + 4 * jj + 5)
                    for h in heads:
                        for nh in range(2):
                            nc.tensor.matmul(
                                vps[h][0:64, nh, :], vatt[:, :, jj, h, 0:64],
                                u[:, :, jj, h, nh, :],
                                start=(jj == 0), stop=(jj == 3), perf_mode=DR)
                        yield 450
                for h in heads:
                    vp = vps[h]
                    _flush_den(img)
                    rr1 = sb.tile([1, HW], f32, name="rr1", tag="rr1")
                    dbc = sb.tile([32, HW], f32, name="dbc", tag="dbc")
                    nc.vector.reciprocal_approx_fast(
                        rr1[0:1, :], vp[0:1, 0:2, :])
                    nc.gpsimd.partition_broadcast(dbc[:, :], rr1[0:1, :])
                    s["pden"] = (vp, dbc, h)
                    yield 600
                return
            for h in heads:
                vp = ps.tile([128, 2, 512], f32, name="mm", tag="mm")
                for jj in range(4):
                    if fence_base is not None:
                        yield ('fence', fence_base + 4 * jj + 5)
                    for nh in range(2):
                        nc.tensor.matmul(
                            vp[0:64, nh, :], vatt[:, :, jj, h, 0:64],
                            u[:, :, jj, h, nh, :],
                            start=(jj == 0), stop=(jj == 3), perf_mode=DR)
                    yield 450
                _flush_den(img)
                rr1 = sb.tile([1, HW], f32, name="rr1", tag="rr1")
                dbc = sb.tile([32, HW], f32, name="dbc", tag="dbc")
                nc.vector.reciprocal_approx_fast(
                    rr1[0:1, :], vp[0:1, 0:2, :])
                nc.gpsimd.partition_broadcast(dbc[:, :], rr1[0:1, :])
                s["pden"] = (vp, dbc, h)
                yield 600

        def phase_P(img):
            """Channel SE + proj (fp8 DR) + residual."""
            s = st[img]
            _flush_den(img)
            comb, pf, xfc = s["comb"], s["pf"], s["xfc"]
            pb = sb.tile([128, 2], bf16, name="pb", tag="pb")
            nc.vector.tensor_copy(pb[:, :], pf[:, :])
            zp = ps.tile([128, 2, 512], f32, name="mm", tag="mm")
            for kc in range(2):
                nc.tensor.matmul(zp[0:16, 0, 0:1],
                                 cwt["wb"][:, OF_CA1 + kc * 16:OF_CA1 + kc * 16 + 16],
                                 pb[:, kc:kc + 1], start=(kc == 0),
                                 stop=(kc == 1))
            z1r = sb.tile([16, 1], bf16, name="z1r", tag="z1r")
            nc.scalar.activation(z1r[:, :], zp[0:16, 0, 0:1], AF.Relu)
            zp2 = ps.tile([128, 2, 512], f32, name="mm", tag="mm")
            for mc in range(2):
                nc.tensor.matmul(zp2[:, mc, 0:1],
                                 cwt["wb"][0:16, OF_CA2 + mc * 128:OF_CA2 + mc * 128 + 128],
                                 z1r[:, :], start=True, stop=True)
            tse = sb.tile([128, 2], f32, name="tse", tag="tse")
            nc.scalar.activation(tse[:, :], zp2[:, 0:2, 0], AF.Tanh, scale=0.5)
            ca = sb.tile([128, 2], f32, name="ca", tag="ca")
            nc.vector.tensor_scalar(ca[:, :], tse[:, :], 0.5, 0.5,
                                    OP.mult, OP.add)
            wps = sb.tile([128, 2, 256], f8, name="wps", tag="wps")
            for kc in range(2):
                nc.vector.tensor_scalar_mul(
                    wps[:, kc, :],
                    cwt["wb"][:, OF_PJ + kc * 256:OF_PJ + kc * 256 + 256],
                    ca[:, kc:kc + 1])
            yield 500
            for mc in range(2):
                pp = ps.tile([128, 2, 512], f32, name="mm", tag="mm")
                for nh in range(2):
                    nc.tensor.matmul(pp[:, nh, :],
                                     wps[:, :, mc * 128:mc * 128 + 128],
                                     comb[:, :, nh * 512:nh * 512 + 512],
                                     start=True, stop=True, perf_mode=DR)
                ot = sb.tile([128, HW], bf16, name="o", tag="o")
                nc.vector.scalar_tensor_tensor(
                    ot.rearrange("p (a b) -> p a b", a=2), pp[:, 0:2, :],
                    0.125, xfc[mc].rearrange("p (a b) -> p a b", a=2),
                    OP.mult, OP.add)
                nc.sync.dma_start(out_d[img, mc], ot[:, :])
                yield 700

        def run(gen):
            for _ in gen:
                pass

        def chain(*gens):
            for g in gens:
                yield from g

        class Pacing:
            """Shared clocks: pacer windows interleave with ONE continuous
            filler stream; filler is never drained at a window boundary.
            Filler items may be ('fence', n): the filler then stalls until
            the pacer has issued n units (issue-order safety)."""
            def __init__(self, filler):
                self.filler = filler
                self.tf = 0.0
                self.tp = 0.0
                self.units = 0
                self.fence = None
                self.alive = True

            def advance_filler(self, ignore_fences=False):
                if not self.alive:
                    return False
                if self.fence is not None:
                    if not ignore_fences and self.units < self.fence:
                        return False
                    self.fence = None
                try:
                    v = next(self.filler)
                except StopIteration:
                    self.alive = False
                    return False
                if isinstance(v, tuple):
                    self.fence = v[1]
                else:
                    self.tf += v
                return True

            def window(self, pacer, pacer_pe=300):
                for cost in pacer:
                    self.tp += cost
                    self.tf += pacer_pe
                    self.units += 1
                    while self.tf < self.tp and self.advance_filler():
                        pass

            def drain(self):
                while self.advance_filler(ignore_fences=True):
                    pass

        run(phase_w1())
        run(phase_x8(0))
        run(phase_wd())
        run(phase_qk(0))
        pac = Pacing(chain(
            phase_gv(0), phase_conv(0),
            phase_x8(1), phase_qk(1), phase_gv(1), phase_xf(0),
            phase_V(0, (0, 1), fence_base=0),
            phase_V(0, (2, 3), fence_base=16), phase_P(0),
            phase_conv(1, blks=(0,), acts=False), phase_conv(1, blks=(1, 2)),
            phase_xf(1), phase_V(1, (0, 1), fence_base=32),
            phase_V(1, (2, 3), fence_base=48, jj_major=True), phase_P(1)))
        pac.window(phase_S(0, (0, 1)))
        pac.window(phase_S(1, (0,)))
        pac.window(phase_S(1, (1,)))
        pac.drain()

    nc.compile()
    return nc


def _prep_weights(i):
    """Host-side packing -> (w8 fp8, wb bf16, wf f32, dwd bf16)."""
    w8 = np.zeros((128, W8), np.float32)
    wb = np.zeros((128, WB), np.float32)
    wf = np.zeros((128, WF), np.float32)

    qkv_w = i["qkv_w"]          # [384, 256]
    for kc in range(2):
        w8[:, OF_QKV + kc * 384:OF_QKV + (kc + 1) * 384] = \
            16.0 * qkv_w[:, kc * 128:(kc + 1) * 128].T
        w8[:, OF_GQ + kc * 128:OF_GQ + (kc + 1) * 128] = \
            16.0 * i["gq_w"][:, kc * 128:(kc + 1) * 128].T
        w8[:, OF_GK + kc * 128:OF_GK + (kc + 1) * 128] = \
            16.0 * i["gkv_w"][0:128, kc * 128:(kc + 1) * 128].T
        w8[:, OF_GV + kc * 128:OF_GV + (kc + 1) * 128] = \
            16.0 * i["gkv_w"][128:256, kc * 128:(kc + 1) * 128].T

    wb[:, OF_A1:OF_A1 + 128] = (i["act_w1"] / 256.0).T
    wb[:, OF_A2:OF_A2 + 128] = (0.5 * i["act_w2"]).T
    for kc in range(2):
        wb[:, OF_CA1 + kc * 16:OF_CA1 + (kc + 1) * 16] = \
            (i["ca_w1"][:, kc * 128:(kc + 1) * 128] / (HW * 16.0)).T
    wb[0:16, OF_CA2:OF_CA2 + 256] = i["ca_w2"].T[0:16, :]
    wb[0:1, OF_GVB:OF_GVB + 128] = 16.0 * i["gkv_b"][128:256]
    for kc in range(2):
        wb[:, OF_PJ + kc * 256:OF_PJ + (kc + 1) * 256] = \
            (0.5 * i["proj_w"][:, kc * 128:(kc + 1) * 128]).T

    dw = i["dw_w"].reshape(QKV_CH, 3, 3)
    dwd = np.zeros((128, 27, 128), np.float32)
    di = np.arange(128)
    for blk in range(3):
        wf[:, OF_QKVB + blk] = i["qkv_b"][blk * 128:(blk + 1) * 128]
        wf[:, OF_DWB + blk] = 16.0 * i["dw_b"][blk * 128:(blk + 1) * 128]
        for tap in range(9):
            dwd[di, blk * 9 + tap, di] = \
                16.0 * dw[blk * 128:(blk + 1) * 128, tap // 3, tap % 3]
    wf[:, OF_B1H] = 0.5 * i["act_b1"]
    wf[:, OF_B2S] = SCALOR * i["act_b2"]
    wf[:, OF_GQB] = i["gq_b"]
    wf[:, OF_GKB] = i["gkv_b"][0:128]
    wf[:, OF_B1] = i["act_b1"]

    return w8.astype(F8), wb.astype(BF16), wf, dwd.astype(BF16)


def _prep_inputs(i):
    """Build the per-core input maps from full inputs."""
    w8, wb, wf, dwd = _prep_weights(i)
    x = i["x"].reshape(16, 256, HW)
    xpb = x + i["proj_b"][None, :, None]          # residual + proj bias fold
    in_maps = []
    for c in range(N_CORES):
        xs = x[c * B_PER_CORE:(c + 1) * B_PER_CORE]       # [2, 256, HW]
        xb = xs.reshape(2, 2, 128, HW).transpose(0, 2, 1, 3)  # [2,128,kc2,HW]
        xf = xpb[c * B_PER_CORE:(c + 1) * B_PER_CORE].reshape(2, 2, 128, HW)
        in_maps.append({
            "xf": np.ascontiguousarray(xf.astype(BF16)),
            "x8": np.ascontiguousarray(xb.astype(F8)),
            "w8": w8, "wb": wb, "wf": wf, "dwd": dwd,
        })
    return in_maps


def kernel(**inputs):
    from concourse.bass_utils import run_bass_kernel_spmd

    i = {k: np.asarray(v, np.float32) for k, v in inputs.items()}
    if "nc" not in _CACHE:
        _CACHE["nc"] = _build_nc()
    nc = _CACHE["nc"]

    in_maps = _prep_inputs(i)
    res = run_bass_kernel_spmd(nc, in_maps, core_ids=list(range(N_CORES)))
    out = np.stack([np.asarray(r["out"], np.float32) for r in res.results])
    return out.reshape(16, 256, 32, 32)
